# revision 30
# baseline (speedup 1.0000x reference)
"""ConvLoRA fused kernel, v6: 1D Winograd F(4,3) along W, fp16, flat DVE ops.

out = conv(x, W_b) + b + conv(x, wA) + conv(x, wB) = conv(x, W_b+wA+wB) + b
(linear in weights, shared input).  F(4,3): 4 output cols per tile, 2x fewer
tensor MACs than direct.  Host packs x segment-major (x_loc[b,j,s,c,y,t],
col 4t+s -> seg s) so all six taps d0..d5 are FLAT shifted views -> every
V-transform op is a contiguous 1D AP (DVE 2x/4x fast modes need packed flat
APs; scalar_tensor_tensor has no fast mode, so the ladder is tensor_scalar +
tensor_tensor only).  Fused weights generated on-device G-transformed; the
reference's reshape maps the LoRA kernel as cin_c = 3*cin + kh - 256*a,
handled by the zero-padded ASE buffer (stride-18 reads, out-of-segment ->
zeros).  Inverse A^T + bias: PSUM evac on DVE (fp16 intermediates), scalar
multiplies + stride-4 output interleave on ACT (DVE strided writes are 3.5x
slower than ACT's).
"""
import sys
sys.path.insert(0, '/opt/trn_rl_repo')
import numpy as np

import concourse.bacc as bacc
import concourse.mybir as mybir
import concourse.tile as tile
from concourse.bass_utils import run_bass_kernel_spmd

f32 = mybir.dt.float32
f16 = mybir.dt.float16
AF = mybir.ActivationFunctionType
OP = mybir.AluOpType

B, CIN, COUT, KS, H, W, R = 16, 256, 256, 3, 128, 128, 16
NCORES = 8
NB = B // NCORES
NSLAB = 4            # 32 output rows per slab
SROWS = 34           # input rows per slab (incl halo)
NTC = W // 4         # 32 winograd tile-cols (4 output cols each)
PITCH = 132          # 4 interleaved segments of 33 (col 4t+s -> seg s)
FV = SROWS * NTC     # 1088 elements per compact V tile
IW = 6               # winograd input taps
SEG = CIN * IW       # 1536: A'' segment length
I_ORDER = (1, 2, 0, 3, 4, 5)   # psum fill order = inverse-transform want order


def _build_nc():
    nc = bacc.Bacc("TRN2", target_bir_lowering=False, debug=False, num_devices=NCORES)

    x_loc = nc.dram_tensor("x_loc", [NB, 2, 128, H + 2, PITCH], f16, kind="ExternalInput")
    wm = nc.dram_tensor("wm", [33, 2 * NB], f32, kind="ExternalInput")
    ew1 = nc.dram_tensor("ew1", [33, 256], f32, kind="ExternalInput")
    w2t = nc.dram_tensor("w2t", [128, 512], f32, kind="ExternalInput")
    b2x = nc.dram_tensor("b2x", [16, 2 * 16 * NB], f32, kind="ExternalInput")
    loraAp = nc.dram_tensor("loraAp", [32, SEG], f16, kind="ExternalInput")
    lba = nc.dram_tensor("lba", [16, 3, 256], f16, kind="ExternalInput")
    wbasep = nc.dram_tensor("wbasep", [128, 18, 2, 256], f16, kind="ExternalInput")
    convb = nc.dram_tensor("convb", [128, 2], f32, kind="ExternalInput")
    out = nc.dram_tensor("out", [NB, COUT, H, W], f32, kind="ExternalOutput")

    with tile.TileContext(nc) as tc:
        from contextlib import ExitStack
        with ExitStack() as ctx:
            cpools = ctx.enter_context(tc.tile_pool(name="consts", bufs=1))
            w24pool = ctx.enter_context(tc.tile_pool(name="w24", bufs=18 * NB))
            xpool = ctx.enter_context(tc.tile_pool(name="xslab", bufs=3))
            vpool = ctx.enter_context(tc.tile_pool(name="vslab", bufs=26))
            vtmp = ctx.enter_context(tc.tile_pool(name="vtmp", bufs=1))

            xs = [None, None]   # current x slab tiles (per chunk)
            vt = [[None, None] for _ in range(IW)]  # V[i][j] current slab

            def load_slab(bi, s, split):
                r0 = 32 * s
                for j in range(2):
                    xx = xpool.tile([128, SROWS * PITCH], f16, tag="xslab")
                    xr = xx[:].rearrange("p (a b) -> p a b", b=PITCH)
                    if split:
                        nc.sync.dma_start(xr[:, 0:18, :], x_loc[bi, j, :, r0:r0 + 18, :])
                        nc.sync.dma_start(xr[:, 18:SROWS, :],
                                          x_loc[bi, j, :, r0 + 18:r0 + SROWS, :])
                    else:
                        nc.sync.dma_start(xr[:, :, :], x_loc[bi, j, :, r0:r0 + SROWS, :])
                    xs[j] = xx

            def _dviews(j):
                xsr = xs[j][:].rearrange("p (a s t) -> p a s t", s=4, t=33)
                d0 = xsr[:, :, 0, 0:NTC]
                d1 = xsr[:, :, 1, 0:NTC]
                d2 = xsr[:, :, 2, 0:NTC]
                d3 = xsr[:, :, 3, 0:NTC]
                d4 = xsr[:, :, 0, 1:NTC + 1]
                d5 = xsr[:, :, 1, 1:NTC + 1]
                return d0, d1, d2, d3, d4, d5

            def _vtile(i, j):
                v = vpool.tile([128, FV], f16, tag="vslab")
                vt[i][j] = v
                return v

            _tmp_n = [0]

            def _tmp(tag, bufs):
                _tmp_n[0] += 1
                return vtmp.tile([128, FV], f16, name=f"vt{_tmp_n[0]}",
                                 tag=tag, bufs=bufs)

            vfeed = [[None] * 5 for _ in range(2)]  # a,b,c,e,f per chunk

            def _r2(t):
                return t[:].rearrange("p (a b) -> p a b", b=NTC)

            def _feed_specs(j):
                d0, d1, d2, d3, d4, d5 = _dviews(j)
                return [(d1, d2, OP.add), (d3, d4, OP.add),
                        (d1, d2, OP.subtract), (d4, d3, OP.subtract),
                        (d1, d3, OP.subtract)]

            def v_gpsimd(_, first=False):
                # strided-src feeders on Pool: a=d1+d2, b=d3+d4, c=d1-d2,
                # e=d4-d3, f=d1-d3.  On the first slab the j=1 chunk goes to
                # DVE so startup isn't serialized on the slow Pool engine.
                for j in range(1 if first else 2):
                    for k, (s0, s1, op) in enumerate(_feed_specs(j)):
                        tr = _tmp("vtmpg", 7)
                        nc.gpsimd.tensor_tensor(_r2(tr)[:], s0, s1, op)
                        vfeed[j][k] = tr

            def v_feed_dve(j):
                for k, (s0, s1, op) in enumerate(_feed_specs(j)):
                    tr = _tmp("vtmpg", 7)
                    nc.vector.tensor_tensor(_r2(tr)[:], s0, s1, op)
                    vfeed[j][k] = tr

            def v_vector_chunk(j):
                # g=d4-d2, h0=d0-d2, h5=d3-d5 strided on DVE; scales on
                # ACT/DVE; finals flat, ordered by conv consumption:
                # V1=-4a+b, V2=4c+e, V0=4h0+g, V3=g-2f, V4=2f+g, V5=4f-h5
                if True:
                    d0, d1, d2, d3, d4, d5 = _dviews(j)
                    ar, br, cr, er, fr = vfeed[j]
                    a4 = _tmp("vtmpv", 8)
                    nc.vector.tensor_scalar_mul(a4[:], ar[:], -4.0)
                    nc.vector.tensor_tensor(_vtile(1, j)[:], a4[:], br[:], OP.add)
                    c4 = _tmp("vtmpv", 8)
                    nc.vector.tensor_scalar_mul(c4[:], cr[:], 4.0)
                    nc.vector.tensor_tensor(_vtile(2, j)[:], c4[:], er[:], OP.add)
                    gr = _tmp("vtmpv", 8)
                    nc.vector.tensor_tensor(_r2(gr)[:], d4, d2, OP.subtract)
                    h0 = _tmp("vtmpv", 8)
                    nc.vector.tensor_tensor(_r2(h0)[:], d0, d2, OP.subtract)
                    h04 = _tmp("vtmpv", 8)
                    nc.scalar.activation(h04[:], h0[:], AF.Copy, scale=4.0)
                    nc.vector.tensor_tensor(_vtile(0, j)[:], h04[:], gr[:], OP.add)
                    f2 = _tmp("vtmpv", 8)
                    nc.scalar.activation(f2[:], fr[:], AF.Copy, scale=2.0)
                    nc.vector.tensor_tensor(_vtile(3, j)[:], gr[:], f2[:], OP.subtract)
                    nc.vector.tensor_tensor(_vtile(4, j)[:], f2[:], gr[:], OP.add)
                    f4 = _tmp("vtmpv", 8)
                    nc.scalar.activation(f4[:], fr[:], AF.Copy, scale=4.0)
                    h5 = _tmp("vtmpv", 8)
                    nc.vector.tensor_tensor(_r2(h5)[:], d3, d5, OP.subtract)
                    nc.vector.tensor_tensor(_vtile(5, j)[:], f4[:], h5[:], OP.subtract)

            def v_vector(_):
                v_vector_chunk(0)
                v_vector_chunk(1)

            w24 = [[[None] * 2 for _ in range(18)] for _ in range(NB)]

            # ---- wgen scope: MLP consts + ASE + wbasep live only until both
            # samples' fused weights are emitted ----
            with ExitStack() as actx:
                wgp = actx.enter_context(tc.tile_pool(name="wgp", bufs=1))
                mlp_sb = actx.enter_context(tc.tile_pool(name="mlpw", bufs=1))

                ASE = []
                for bi in range(NB):
                    a = wgp.tile([16, 3 * SEG], f16, tag="ase", bufs=NB)
                    nc.gpsimd.memset(a[:].bitcast(f32), 0.0)
                    ASE.append(a)

                wm_sb = wgp.tile([33, 2 * NB], f32)
                nc.sync.dma_start(wm_sb[:], wm[:])
                ew1_sb = wgp.tile([33, 256], f32)
                nc.sync.dma_start(ew1_sb[:], ew1[:])
                load_slab(0, 0, split=True)
                w2t_sb = wgp.tile([128, 512], f32)
                nc.sync.dma_start(w2t_sb[:], w2t[:])
                b2x_sb = wgp.tile([16, 2 * 16 * NB], f32)
                nc.sync.dma_start(b2x_sb[:], b2x[:])
                lba_sb = wgp.tile([16, 3, 256], f16)
                nc.sync.dma_start(lba_sb[:], lba[:])
                convb_sb = cpools.tile([128, 2], f32)
                nc.sync.dma_start(convb_sb[:], convb[:])
                wbasep_sb = wgp.tile([128, 18, 2, 256], f16)
                for p0 in range(0, 128, 32):
                    nc.sync.dma_start(wbasep_sb[p0:p0 + 32, 0:9], wbasep[p0:p0 + 32, 0:9])
                for p0 in range(0, 128, 32):
                    nc.sync.dma_start(wbasep_sb[p0:p0 + 32, 9:18], wbasep[p0:p0 + 32, 9:18])

                # ---- MLP + A'' for both samples ----
                coff_sb = []
                with ExitStack() as mctx:
                    ps_h = mctx.enter_context(tc.tile_pool(name="psh", bufs=1, space="PSUM"))
                    ps_c = mctx.enter_context(tc.tile_pool(name="psc", bufs=2, space="PSUM"))
                    ps_a = mctx.enter_context(tc.tile_pool(name="psa", bufs=1, space="PSUM"))

                    loraA1_sb = mlp_sb.tile([16, SEG], f16, tag="la", bufs=2)
                    loraA2_sb = mlp_sb.tile([16, SEG], f16, tag="la", bufs=2)
                    loraA_sb = [loraA1_sb, loraA2_sb]
                    nc.sync.dma_start(loraA_sb[0][:], loraAp[0:16, :])
                    nc.sync.dma_start(loraA_sb[1][:], loraAp[16:32, :])

                    haug = mlp_sb.tile([128, 2 * NB], f32)
                    for br in range(2):
                        h_ps = ps_h.tile([128, NB], f32, tag="hps")
                        nc.tensor.matmul(h_ps[:], ew1_sb[:, 128 * br:128 * (br + 1)],
                                         wm_sb[:, NB * br:NB * (br + 1)], start=True, stop=True)
                        h_sb = mlp_sb.tile([128, NB], f32, tag="hsb", bufs=2)
                        nc.scalar.activation(h_sb[:], h_ps[:], AF.Copy)
                        nc.vector.scalar_tensor_tensor(
                            haug[:, NB * br:NB * (br + 1)], h_sb[:], 0.2, h_sb[:],
                            OP.mult, OP.max)
                    b2x_r = b2x_sb[:, :].rearrange("q (br r b) -> q br r b", br=2, b=NB)
                    for br in range(2):
                        c_ps = ps_c.tile([16, 16, NB], f32, tag="cps2")
                        for r in range(16):
                            nc.tensor.matmul(c_ps[:, r, :],
                                             w2t_sb[:, 256 * br + 16 * r:256 * br + 16 * (r + 1)],
                                             haug[:, NB * br:NB * (br + 1)],
                                             start=True, stop=True)
                        csb = wgp.tile([16, 16, NB], f16, tag="coff", bufs=2)
                        nc.vector.tensor_add(csb[:], c_ps[:], b2x_r[:, br])
                        coff_sb.append(csb)
                    for bi in range(NB):
                        a_ps = ps_a.tile([16, SEG], f32, tag="aps")
                        for c0 in range(0, SEG, 512):
                            nc.tensor.matmul(a_ps[:, c0:c0 + 512], coff_sb[0][:, :, bi],
                                             loraA_sb[0][:, c0:c0 + 512], start=True, stop=False)
                            nc.tensor.matmul(a_ps[:, c0:c0 + 512], coff_sb[1][:, :, bi],
                                             loraA_sb[1][:, c0:c0 + 512], start=False, stop=True)
                        nc.vector.tensor_copy(ASE[bi][:, SEG:2 * SEG], a_ps[:])

                cps = ctx.enter_context(tc.tile_pool(name="cps", bufs=8, space="PSUM"))

                def emit_wgen(bi):
                    # both j-chunks share one 512-wide psum -> one vector add
                    # per tap (halves tensor<->vector psum-recycle round-trips)
                    ase_r = ASE[bi][:].rearrange("p (c n) -> p c n", n=18)
                    for w_i, i in enumerate(I_ORDER):
                        for kh in range(3):
                            tp = 6 * kh + i
                            wg = cps.tile([128, 512], f32, tag="cps")
                            for j in range(2):
                                for idx, a in enumerate((j, j + 1)):
                                    base = SEG + 2304 * j + tp - SEG * a
                                    c0, n0 = divmod(base, 18)
                                    nc.tensor.matmul(wg[:, 256 * j:256 * (j + 1)],
                                                     ase_r[:, c0:c0 + 128, n0],
                                                     lba_sb[:, a, :],
                                                     start=(idx == 0), stop=(idx == 1))
                            wt = w24pool.tile([128, 512], f16, tag="w24")
                            nc.vector.tensor_add(
                                wt[:], wg[:],
                                wbasep_sb[:, tp].rearrange("p a b -> p (a b)"))
                            w24[bi][tp][0] = wt
                            w24[bi][tp][1] = wt

                v_gpsimd(None, first=True)
                v_feed_dve(1)
                v_vector(None)
                emit_wgen(0)
                emit_wgen(1)

            # ---- conv (stg pools open after wgen scope frees its SBUF) ----
            bctx = ctx.enter_context(ExitStack())
            stg = bctx.enter_context(tc.tile_pool(name="stg", bufs=2))
            stg2 = bctx.enter_context(tc.tile_pool(name="stg2", bufs=1))

            def conv_slab(bi, s, oc, cvt, between=None, rev=False):
                for p2 in ((1, 0) if rev else (0, 1)):
                    psA = {}
                    for i in I_ORDER:
                        ps = cps.tile([128, 512], f32, tag="cps")
                        w = 0
                        for kh in range(3):
                            for j in range(2):
                                c0 = NTC * (16 * p2 + kh)
                                nc.tensor.matmul(
                                    ps[:],
                                    w24[bi][6 * kh + i][j][:, 256 * j + 128 * oc:
                                                           256 * j + 128 * (oc + 1)],
                                    cvt[i][j][:, c0:c0 + 512],
                                    start=(w == 0), stop=(w == 5))
                                w += 1
                        psA[i] = ps
                    # inverse: u=m1+m2, v=m1-m2, p=m3+m4, q=m3-m4 (bias in m1);
                    # y0=m0+u+p, y1=v+2q, y2=u+4p, y3=v+8q+m5.  PSUM readers on
                    # DVE (f32 in, fp16 out), scalar mults + interleave on ACT.
                    m1s = stg2.tile([128, 512], f32, tag="mst", bufs=5)
                    nc.scalar.activation(m1s[:], psA[1][:], AF.Identity,
                                         bias=convb_sb[:, oc:oc + 1])
                    u_ = stg2.tile([128, 512], f16, tag="stg2", bufs=14)
                    nc.vector.tensor_tensor(u_[:], m1s[:], psA[2][:], OP.add)
                    v_ = stg2.tile([128, 512], f16, tag="stg2", bufs=14)
                    nc.vector.tensor_tensor(v_[:], m1s[:], psA[2][:], OP.subtract)
                    m3s = stg2.tile([128, 512], f32, tag="mst", bufs=5)
                    nc.scalar.activation(m3s[:], psA[3][:], AF.Copy)
                    p_ = stg2.tile([128, 512], f16, tag="stg2", bufs=14)
                    nc.vector.tensor_tensor(p_[:], m3s[:], psA[4][:], OP.add)
                    q_ = stg2.tile([128, 512], f16, tag="stg2", bufs=14)
                    nc.vector.tensor_tensor(q_[:], m3s[:], psA[4][:], OP.subtract)
                    t0 = stg2.tile([128, 512], f16, tag="stg2", bufs=14)
                    nc.vector.tensor_tensor(t0[:], psA[0][:], u_[:], OP.add)
                    q2 = stg2.tile([128, 512], f16, tag="stg2", bufs=14)
                    nc.scalar.activation(q2[:], q_[:], AF.Copy, scale=2.0)
                    p4 = stg2.tile([128, 512], f16, tag="stg2", bufs=14)
                    nc.scalar.activation(p4[:], p_[:], AF.Copy, scale=4.0)
                    q8 = stg2.tile([128, 512], f16, tag="stg2", bufs=14)
                    nc.scalar.activation(q8[:], q_[:], AF.Copy, scale=8.0)
                    yc = []
                    for _d in range(4):
                        yt = stg2.tile([128, 512], f16, name=f"yc{_d}", tag="yc", bufs=8)
                        yc.append(yt)
                    nc.vector.tensor_tensor(yc[0][:], t0[:], p_[:], OP.add)
                    nc.vector.tensor_tensor(yc[1][:], v_[:], q2[:], OP.add)
                    nc.vector.tensor_tensor(yc[2][:], u_[:], p4[:], OP.add)
                    t3 = stg2.tile([128, 512], f16, tag="stg2", bufs=14)
                    nc.vector.tensor_tensor(t3[:], v_[:], q8[:], OP.add)
                    nc.vector.tensor_tensor(yc[3][:], t3[:], psA[5][:], OP.add)
                    st = stg.tile([128, 16, 128], f32, tag="stg")
                    str_ = st[:].rearrange("p a (b c) -> p a b c", c=4)
                    for dd in range(4):
                        ycr = yc[dd][:].rearrange("p (a b) -> p a b", b=NTC)
                        nc.scalar.activation(str_[:, :, :, dd], ycr[:], AF.Copy)
                    y0 = 32 * s + 16 * p2
                    nc.sync.dma_start(
                        out[bi, 128 * oc:128 * (oc + 1), y0:y0 + 16, :], st[:])
                    if p2 == 0 and between is not None:
                        between()

            for bi in range(NB):
                for s in range(NSLAB):
                    nxt = (bi, s + 1) if s + 1 < NSLAB else \
                          ((bi + 1, 0) if bi + 1 < NB else None)
                    if nxt:
                        load_slab(nxt[0], nxt[1], split=False)
                    cvt = [[vt[i][0], vt[i][1]] for i in range(IW)]
                    conv_slab(bi, s, 0, cvt)
                    if nxt:
                        v_gpsimd(nxt)
                        v_vector(nxt)
                    conv_slab(bi, s, 1, cvt, rev=(nxt is None))
    nc.finalize()
    return nc


G43 = np.array([[1 / 4, 0, 0], [-1 / 6, -1 / 6, -1 / 6], [-1 / 6, 1 / 6, -1 / 6],
                [1 / 24, 1 / 12, 1 / 6], [1 / 24, -1 / 12, 1 / 6], [0, 0, 1]],
               np.float64)


def _host_prep(inputs):
    x = np.asarray(inputs["x"], dtype=np.float32)
    wms = np.asarray(inputs["wms"], dtype=np.float32)
    conv_w = np.asarray(inputs["conv_w"], dtype=np.float32)
    conv_b = np.asarray(inputs["conv_b"], dtype=np.float32)
    e_w1 = [np.asarray(inputs["e1_w1"], np.float32), np.asarray(inputs["e2_w1"], np.float32)]
    e_b1 = [np.asarray(inputs["e1_b1"], np.float32), np.asarray(inputs["e2_b1"], np.float32)]
    e_w2 = [np.asarray(inputs["e1_w2"], np.float32), np.asarray(inputs["e2_w2"], np.float32)]
    e_b2 = [np.asarray(inputs["e1_b2"], np.float32), np.asarray(inputs["e2_b2"], np.float32)]
    lora_A = [np.asarray(inputs["lora_A1"], np.float32), np.asarray(inputs["lora_A2"], np.float32)]
    lora_B = np.asarray(inputs["lora_B"], np.float32)

    ew1 = np.zeros((33, 256), np.float32)
    for br in range(2):
        ew1[:32, 128 * br:128 * (br + 1)] = e_w1[br].T
        ew1[32, 128 * br:128 * (br + 1)] = e_b1[br]
    w2t = np.concatenate([e_w2[0].T, e_w2[1].T], axis=1).astype(np.float32)
    b2x = np.zeros((16, 2, 16, NB), np.float32)
    for br in range(2):
        b2x[:, br, :, :] = e_b2[br].reshape(16, 16).T[:, :, None]
    b2x = np.ascontiguousarray(b2x.reshape(16, 2 * 16 * NB))
    # G-folded A': A'[q, 6T+i] = sum_kw G43[i,kw] A[q, 3T+kw]
    def gfold(A):
        return np.einsum('ik,qtk->qti', G43, A.reshape(R, 256, 3).astype(np.float64)
                         ).reshape(R, SEG)
    loraAp = np.concatenate([gfold(lora_A[0]), gfold(lora_A[1])], 0).astype(np.float16)
    lba = np.ascontiguousarray(lora_B.reshape(256, 3, 16).transpose(2, 1, 0)
                               ).astype(np.float16)
    # wbasep[p, 6kh+i, j, cout] = sum_kw G43[i,kw] conv_w[cout, 128j+p, kh, kw]
    wb = np.einsum('ik,ocnk->nioc', G43, conv_w.astype(np.float64))  # [kh, i, cout, cin]
    wb = wb.reshape(3, 6, 256, 2, 128).transpose(4, 0, 1, 3, 2)     # [p, kh, i, j, cout]
    wbasep = np.ascontiguousarray(wb.reshape(128, 18, 2, 256)).astype(np.float16)
    convb = np.ascontiguousarray(conv_b.reshape(2, 128).T)

    xpad = np.zeros((B, 2, 128, H + 2, PITCH), dtype=np.float16)
    xpad[:, :, :, 1:H + 1, 1:W + 1] = x.reshape(B, 2, 128, H, W).astype(np.float16)
    # pack each padded row into 4 interleaved segments of 33 (col 4t+s -> seg s)
    xp = np.ascontiguousarray(
        xpad.reshape(B, 2, 128, H + 2, 33, 4).transpose(0, 1, 2, 3, 5, 4)
    ).reshape(B, 2, 128, H + 2, PITCH)
    in_maps = []
    for core in range(NCORES):
        b0 = core * NB
        wmc = np.ones((33, 2 * NB), np.float32)
        for br in range(2):
            for bi in range(NB):
                wmc[:32, NB * br + bi] = wms[br, b0 + bi]
        in_maps.append({
            "x_loc": np.ascontiguousarray(xp[b0:b0 + NB]),
            "wm": wmc, "ew1": ew1, "w2t": w2t, "b2x": b2x,
            "loraAp": loraAp, "lba": lba, "wbasep": wbasep, "convb": convb,
        })
    return in_maps


_NC = None


def kernel(**inputs) -> np.ndarray:
    global _NC
    if _NC is None:
        _NC = _build_nc()
    in_maps = _host_prep(inputs)
    res = run_bass_kernel_spmd(_NC, in_maps, core_ids=list(range(NCORES)))
    return np.concatenate([res.results[c]["out"] for c in range(NCORES)], axis=0)


# revision 31
# speedup vs baseline: 1.0346x; 1.0346x over previous
"""ConvLoRA fused kernel, v6: 1D Winograd F(4,3) along W, fp16, flat DVE ops.

out = conv(x, W_b) + b + conv(x, wA) + conv(x, wB) = conv(x, W_b+wA+wB) + b
(linear in weights, shared input).  F(4,3): 4 output cols per tile, 2x fewer
tensor MACs than direct.  Host packs x segment-major (x_loc[b,j,s,c,y,t],
col 4t+s -> seg s) so all six taps d0..d5 are FLAT shifted views -> every
V-transform op is a contiguous 1D AP (DVE 2x/4x fast modes need packed flat
APs; scalar_tensor_tensor has no fast mode, so the ladder is tensor_scalar +
tensor_tensor only).  Fused weights generated on-device G-transformed; the
reference's reshape maps the LoRA kernel as cin_c = 3*cin + kh - 256*a,
handled by the zero-padded ASE buffer (stride-18 reads, out-of-segment ->
zeros).  Inverse A^T + bias: PSUM evac on DVE (fp16 intermediates), scalar
multiplies + stride-4 output interleave on ACT (DVE strided writes are 3.5x
slower than ACT's).
"""
import sys
sys.path.insert(0, '/opt/trn_rl_repo')
import numpy as np

import concourse.bacc as bacc
import concourse.mybir as mybir
import concourse.tile as tile
from concourse.bass_utils import run_bass_kernel_spmd

f32 = mybir.dt.float32
f16 = mybir.dt.float16
AF = mybir.ActivationFunctionType
OP = mybir.AluOpType

B, CIN, COUT, KS, H, W, R = 16, 256, 256, 3, 128, 128, 16
NCORES = 8
NB = B // NCORES
NSLAB = 4            # 32 output rows per slab
SROWS = 34           # input rows per slab (incl halo)
NTC = W // 4         # 32 winograd tile-cols (4 output cols each)
PITCH = 132          # 4 interleaved segments of 33 (col 4t+s -> seg s)
FV = SROWS * NTC     # 1088 elements per compact V tile
IW = 6               # winograd input taps
SEG = CIN * IW       # 1536: A'' segment length
I_ORDER = (1, 2, 0, 3, 4, 5)   # psum fill order = inverse-transform want order


def _build_nc():
    nc = bacc.Bacc("TRN2", target_bir_lowering=False, debug=False, num_devices=NCORES)

    x_loc = nc.dram_tensor("x_loc", [NB, 2, 128, H + 2, PITCH], f16, kind="ExternalInput")
    wm = nc.dram_tensor("wm", [33, 2 * NB], f32, kind="ExternalInput")
    ew1 = nc.dram_tensor("ew1", [33, 256], f32, kind="ExternalInput")
    w2t = nc.dram_tensor("w2t", [128, 512], f32, kind="ExternalInput")
    b2x = nc.dram_tensor("b2x", [16, 2 * 16 * NB], f32, kind="ExternalInput")
    loraAp = nc.dram_tensor("loraAp", [32, SEG], f16, kind="ExternalInput")
    lba = nc.dram_tensor("lba", [16, 3, 256], f16, kind="ExternalInput")
    wbasep = nc.dram_tensor("wbasep", [128, 18, 2, 256], f16, kind="ExternalInput")
    convb = nc.dram_tensor("convb", [128, 2], f32, kind="ExternalInput")
    out = nc.dram_tensor("out", [NB, COUT, H, W], f32, kind="ExternalOutput")

    with tile.TileContext(nc) as tc:
        from contextlib import ExitStack
        with ExitStack() as ctx:
            cpools = ctx.enter_context(tc.tile_pool(name="consts", bufs=1))
            w24pool = ctx.enter_context(tc.tile_pool(name="w24", bufs=18 * 2 * NB))
            xpool = ctx.enter_context(tc.tile_pool(name="xslab", bufs=3))
            vpool = ctx.enter_context(tc.tile_pool(name="vslab", bufs=26))
            vtmp = ctx.enter_context(tc.tile_pool(name="vtmp", bufs=1))

            xs = [None, None]   # current x slab tiles (per chunk)
            vt = [[None, None] for _ in range(IW)]  # V[i][j] current slab

            def load_slab(bi, s, split):
                r0 = 32 * s
                for j in range(2):
                    xx = xpool.tile([128, SROWS * PITCH], f16, tag="xslab")
                    xr = xx[:].rearrange("p (a b) -> p a b", b=PITCH)
                    if split:
                        nc.sync.dma_start(xr[:, 0:18, :], x_loc[bi, j, :, r0:r0 + 18, :])
                        nc.sync.dma_start(xr[:, 18:SROWS, :],
                                          x_loc[bi, j, :, r0 + 18:r0 + SROWS, :])
                    else:
                        nc.sync.dma_start(xr[:, :, :], x_loc[bi, j, :, r0:r0 + SROWS, :])
                    xs[j] = xx

            def _dviews(j):
                xsr = xs[j][:].rearrange("p (a s t) -> p a s t", s=4, t=33)
                d0 = xsr[:, :, 0, 0:NTC]
                d1 = xsr[:, :, 1, 0:NTC]
                d2 = xsr[:, :, 2, 0:NTC]
                d3 = xsr[:, :, 3, 0:NTC]
                d4 = xsr[:, :, 0, 1:NTC + 1]
                d5 = xsr[:, :, 1, 1:NTC + 1]
                return d0, d1, d2, d3, d4, d5

            def _vtile(i, j):
                v = vpool.tile([128, FV], f16, tag="vslab")
                vt[i][j] = v
                return v

            _tmp_n = [0]

            def _tmp(tag, bufs):
                _tmp_n[0] += 1
                return vtmp.tile([128, FV], f16, name=f"vt{_tmp_n[0]}",
                                 tag=tag, bufs=bufs)

            vfeed = [[None] * 5 for _ in range(2)]  # a,b,c,e,f per chunk

            def _r2(t):
                return t[:].rearrange("p (a b) -> p a b", b=NTC)

            def _feed_specs(j):
                d0, d1, d2, d3, d4, d5 = _dviews(j)
                return [(d1, d2, OP.add), (d3, d4, OP.add),
                        (d1, d2, OP.subtract), (d4, d3, OP.subtract),
                        (d1, d3, OP.subtract)]

            def v_gpsimd(_, first=False):
                # strided-src feeders on Pool: a=d1+d2, b=d3+d4, c=d1-d2,
                # e=d4-d3, f=d1-d3.  On the first slab the j=1 chunk goes to
                # DVE so startup isn't serialized on the slow Pool engine.
                for j in range(1 if first else 2):
                    for k, (s0, s1, op) in enumerate(_feed_specs(j)):
                        tr = _tmp("vtmpg", 7)
                        nc.gpsimd.tensor_tensor(_r2(tr)[:], s0, s1, op)
                        vfeed[j][k] = tr

            def v_feed_dve(j):
                for k, (s0, s1, op) in enumerate(_feed_specs(j)):
                    tr = _tmp("vtmpg", 7)
                    nc.vector.tensor_tensor(_r2(tr)[:], s0, s1, op)
                    vfeed[j][k] = tr

            def v_vector_chunk(j):
                # g=d4-d2, h0=d0-d2, h5=d3-d5 strided on DVE; scales on
                # ACT/DVE; finals flat, ordered by conv consumption:
                # V1=-4a+b, V2=4c+e, V0=4h0+g, V3=g-2f, V4=2f+g, V5=4f-h5
                if True:
                    d0, d1, d2, d3, d4, d5 = _dviews(j)
                    ar, br, cr, er, fr = vfeed[j]
                    a4 = _tmp("vtmpv", 8)
                    nc.vector.tensor_scalar_mul(a4[:], ar[:], -4.0)
                    nc.vector.tensor_tensor(_vtile(1, j)[:], a4[:], br[:], OP.add)
                    c4 = _tmp("vtmpv", 8)
                    nc.vector.tensor_scalar_mul(c4[:], cr[:], 4.0)
                    nc.vector.tensor_tensor(_vtile(2, j)[:], c4[:], er[:], OP.add)
                    gr = _tmp("vtmpv", 8)
                    nc.vector.tensor_tensor(_r2(gr)[:], d4, d2, OP.subtract)
                    h0 = _tmp("vtmpv", 8)
                    nc.vector.tensor_tensor(_r2(h0)[:], d0, d2, OP.subtract)
                    h04 = _tmp("vtmpv", 8)
                    nc.scalar.activation(h04[:], h0[:], AF.Copy, scale=4.0)
                    nc.vector.tensor_tensor(_vtile(0, j)[:], h04[:], gr[:], OP.add)
                    f2 = _tmp("vtmpv", 8)
                    nc.scalar.activation(f2[:], fr[:], AF.Copy, scale=2.0)
                    nc.vector.tensor_tensor(_vtile(3, j)[:], gr[:], f2[:], OP.subtract)
                    nc.vector.tensor_tensor(_vtile(4, j)[:], f2[:], gr[:], OP.add)
                    f4 = _tmp("vtmpv", 8)
                    nc.scalar.activation(f4[:], fr[:], AF.Copy, scale=4.0)
                    h5 = _tmp("vtmpv", 8)
                    nc.vector.tensor_tensor(_r2(h5)[:], d3, d5, OP.subtract)
                    nc.vector.tensor_tensor(_vtile(5, j)[:], f4[:], h5[:], OP.subtract)

            def v_vector(_):
                v_vector_chunk(0)
                v_vector_chunk(1)

            w24 = [[[None] * 2 for _ in range(18)] for _ in range(NB)]

            # ---- wgen scope: MLP consts + ASE + wbasep live only until both
            # samples' fused weights are emitted ----
            with ExitStack() as actx:
                wgp = actx.enter_context(tc.tile_pool(name="wgp", bufs=1))
                mlp_sb = actx.enter_context(tc.tile_pool(name="mlpw", bufs=1))

                ASE = []
                for bi in range(NB):
                    a = wgp.tile([16, 3 * SEG], f16, tag="ase", bufs=NB)
                    nc.gpsimd.memset(a[:].bitcast(f32), 0.0)
                    ASE.append(a)

                wm_sb = wgp.tile([33, 2 * NB], f32)
                nc.sync.dma_start(wm_sb[:], wm[:])
                ew1_sb = wgp.tile([33, 256], f32)
                nc.sync.dma_start(ew1_sb[:], ew1[:])
                load_slab(0, 0, split=True)
                w2t_sb = wgp.tile([128, 512], f32)
                nc.sync.dma_start(w2t_sb[:], w2t[:])
                b2x_sb = wgp.tile([16, 2 * 16 * NB], f32)
                nc.sync.dma_start(b2x_sb[:], b2x[:])
                lba_sb = wgp.tile([16, 3, 256], f16)
                nc.sync.dma_start(lba_sb[:], lba[:])
                convb_sb = cpools.tile([128, 2], f32)
                nc.sync.dma_start(convb_sb[:], convb[:])
                wbasep_sb = wgp.tile([128, 18, 2, 256], f16)
                for p0 in range(0, 128, 32):
                    nc.sync.dma_start(wbasep_sb[p0:p0 + 32, 0:9], wbasep[p0:p0 + 32, 0:9])
                for p0 in range(0, 128, 32):
                    nc.sync.dma_start(wbasep_sb[p0:p0 + 32, 9:18], wbasep[p0:p0 + 32, 9:18])

                # ---- MLP + A'' for both samples ----
                coff_sb = []
                with ExitStack() as mctx:
                    ps_h = mctx.enter_context(tc.tile_pool(name="psh", bufs=1, space="PSUM"))
                    ps_c = mctx.enter_context(tc.tile_pool(name="psc", bufs=2, space="PSUM"))
                    ps_a = mctx.enter_context(tc.tile_pool(name="psa", bufs=1, space="PSUM"))

                    loraA1_sb = mlp_sb.tile([16, SEG], f16, tag="la", bufs=2)
                    loraA2_sb = mlp_sb.tile([16, SEG], f16, tag="la", bufs=2)
                    loraA_sb = [loraA1_sb, loraA2_sb]
                    nc.sync.dma_start(loraA_sb[0][:], loraAp[0:16, :])
                    nc.sync.dma_start(loraA_sb[1][:], loraAp[16:32, :])

                    haug = mlp_sb.tile([128, 2 * NB], f32)
                    for br in range(2):
                        h_ps = ps_h.tile([128, NB], f32, tag="hps")
                        nc.tensor.matmul(h_ps[:], ew1_sb[:, 128 * br:128 * (br + 1)],
                                         wm_sb[:, NB * br:NB * (br + 1)], start=True, stop=True)
                        h_sb = mlp_sb.tile([128, NB], f32, tag="hsb", bufs=2)
                        nc.scalar.activation(h_sb[:], h_ps[:], AF.Copy)
                        nc.vector.scalar_tensor_tensor(
                            haug[:, NB * br:NB * (br + 1)], h_sb[:], 0.2, h_sb[:],
                            OP.mult, OP.max)
                    b2x_r = b2x_sb[:, :].rearrange("q (br r b) -> q br r b", br=2, b=NB)
                    for br in range(2):
                        c_ps = ps_c.tile([16, 16, NB], f32, tag="cps2")
                        for r in range(16):
                            nc.tensor.matmul(c_ps[:, r, :],
                                             w2t_sb[:, 256 * br + 16 * r:256 * br + 16 * (r + 1)],
                                             haug[:, NB * br:NB * (br + 1)],
                                             start=True, stop=True)
                        csb = wgp.tile([16, 16, NB], f16, tag="coff", bufs=2)
                        nc.vector.tensor_add(csb[:], c_ps[:], b2x_r[:, br])
                        coff_sb.append(csb)
                    for bi in range(NB):
                        a_ps = ps_a.tile([16, SEG], f32, tag="aps")
                        for c0 in range(0, SEG, 512):
                            nc.tensor.matmul(a_ps[:, c0:c0 + 512], coff_sb[0][:, :, bi],
                                             loraA_sb[0][:, c0:c0 + 512], start=True, stop=False)
                            nc.tensor.matmul(a_ps[:, c0:c0 + 512], coff_sb[1][:, :, bi],
                                             loraA_sb[1][:, c0:c0 + 512], start=False, stop=True)
                        nc.vector.tensor_copy(ASE[bi][:, SEG:2 * SEG], a_ps[:])

                cps = ctx.enter_context(tc.tile_pool(name="cps", bufs=8, space="PSUM"))

                def emit_wgen(bi):
                    # tp order follows conv consumption (i-groups in I_ORDER)
                    ase_r = ASE[bi][:].rearrange("p (c n) -> p c n", n=18)
                    for w_i, i in enumerate(I_ORDER):
                        for kh in range(3):
                            tp = 6 * kh + i
                            for j in range(2):
                                wg = cps.tile([128, 512], f32, tag="cps")
                                for idx, a in enumerate((j, j + 1)):
                                    base = SEG + 2304 * j + tp - SEG * a
                                    c0, n0 = divmod(base, 18)
                                    nc.tensor.matmul(wg[:, 0:256], ase_r[:, c0:c0 + 128, n0],
                                                     lba_sb[:, a, :],
                                                     start=(idx == 0), stop=(idx == 1))
                                wt = w24pool.tile([128, 256], f16, tag="w24")
                                nc.vector.tensor_add(wt[:], wg[:, 0:256], wbasep_sb[:, tp, j])
                                w24[bi][tp][j] = wt

                v_gpsimd(None)
                v_vector(None)
                emit_wgen(0)
                emit_wgen(1)

            # ---- conv (stg pools open after wgen scope frees its SBUF) ----
            bctx = ctx.enter_context(ExitStack())
            stg = bctx.enter_context(tc.tile_pool(name="stg", bufs=2))
            stg2 = bctx.enter_context(tc.tile_pool(name="stg2", bufs=1))

            def conv_slab(bi, s, oc, cvt, between=None):
                for p2 in range(2):
                    psA = {}
                    for i in I_ORDER:
                        ps = cps.tile([128, 512], f32, tag="cps")
                        w = 0
                        for kh in range(3):
                            for j in range(2):
                                c0 = NTC * (16 * p2 + kh)
                                nc.tensor.matmul(
                                    ps[:],
                                    w24[bi][6 * kh + i][j][:, 128 * oc:128 * (oc + 1)],
                                    cvt[i][j][:, c0:c0 + 512],
                                    start=(w == 0), stop=(w == 5))
                                w += 1
                        psA[i] = ps
                    # inverse: u=m1+m2, v=m1-m2, p=m3+m4, q=m3-m4 (bias in m1);
                    # y0=m0+u+p, y1=v+2q, y2=u+4p, y3=v+8q+m5.  PSUM readers on
                    # DVE (f32 in, fp16 out), scalar mults + interleave on ACT.
                    m1s = stg2.tile([128, 512], f32, tag="mst", bufs=5)
                    nc.scalar.activation(m1s[:], psA[1][:], AF.Identity,
                                         bias=convb_sb[:, oc:oc + 1])
                    u_ = stg2.tile([128, 512], f16, tag="stg2", bufs=14)
                    nc.vector.tensor_tensor(u_[:], m1s[:], psA[2][:], OP.add)
                    v_ = stg2.tile([128, 512], f16, tag="stg2", bufs=14)
                    nc.vector.tensor_tensor(v_[:], m1s[:], psA[2][:], OP.subtract)
                    m3s = stg2.tile([128, 512], f32, tag="mst", bufs=5)
                    nc.scalar.activation(m3s[:], psA[3][:], AF.Copy)
                    p_ = stg2.tile([128, 512], f16, tag="stg2", bufs=14)
                    nc.vector.tensor_tensor(p_[:], m3s[:], psA[4][:], OP.add)
                    q_ = stg2.tile([128, 512], f16, tag="stg2", bufs=14)
                    nc.vector.tensor_tensor(q_[:], m3s[:], psA[4][:], OP.subtract)
                    t0 = stg2.tile([128, 512], f16, tag="stg2", bufs=14)
                    nc.vector.tensor_tensor(t0[:], psA[0][:], u_[:], OP.add)
                    q2 = stg2.tile([128, 512], f16, tag="stg2", bufs=14)
                    nc.scalar.activation(q2[:], q_[:], AF.Copy, scale=2.0)
                    p4 = stg2.tile([128, 512], f16, tag="stg2", bufs=14)
                    nc.scalar.activation(p4[:], p_[:], AF.Copy, scale=4.0)
                    q8 = stg2.tile([128, 512], f16, tag="stg2", bufs=14)
                    nc.scalar.activation(q8[:], q_[:], AF.Copy, scale=8.0)
                    yc = []
                    for _d in range(4):
                        yt = stg2.tile([128, 512], f16, name=f"yc{_d}", tag="yc", bufs=8)
                        yc.append(yt)
                    nc.vector.tensor_tensor(yc[0][:], t0[:], p_[:], OP.add)
                    nc.vector.tensor_tensor(yc[1][:], v_[:], q2[:], OP.add)
                    nc.vector.tensor_tensor(yc[2][:], u_[:], p4[:], OP.add)
                    t3 = stg2.tile([128, 512], f16, tag="stg2", bufs=14)
                    nc.vector.tensor_tensor(t3[:], v_[:], q8[:], OP.add)
                    nc.vector.tensor_tensor(yc[3][:], t3[:], psA[5][:], OP.add)
                    st = stg.tile([128, 16, 128], f32, tag="stg")
                    str_ = st[:].rearrange("p a (b c) -> p a b c", c=4)
                    for dd in range(4):
                        ycr = yc[dd][:].rearrange("p (a b) -> p a b", b=NTC)
                        nc.scalar.activation(str_[:, :, :, dd], ycr[:], AF.Copy)
                    y0 = 32 * s + 16 * p2
                    nc.sync.dma_start(
                        out[bi, 128 * oc:128 * (oc + 1), y0:y0 + 16, :], st[:])
                    if p2 == 0 and between is not None:
                        between()

            for bi in range(NB):
                for s in range(NSLAB):
                    nxt = (bi, s + 1) if s + 1 < NSLAB else \
                          ((bi + 1, 0) if bi + 1 < NB else None)
                    if nxt:
                        load_slab(nxt[0], nxt[1], split=False)
                    cvt = [[vt[i][0], vt[i][1]] for i in range(IW)]
                    conv_slab(bi, s, 0, cvt)
                    if nxt:
                        v_gpsimd(nxt)
                        v_vector(nxt)
                    conv_slab(bi, s, 1, cvt)
    nc.finalize()
    return nc


G43 = np.array([[1 / 4, 0, 0], [-1 / 6, -1 / 6, -1 / 6], [-1 / 6, 1 / 6, -1 / 6],
                [1 / 24, 1 / 12, 1 / 6], [1 / 24, -1 / 12, 1 / 6], [0, 0, 1]],
               np.float64)


def _host_prep(inputs):
    x = np.asarray(inputs["x"], dtype=np.float32)
    wms = np.asarray(inputs["wms"], dtype=np.float32)
    conv_w = np.asarray(inputs["conv_w"], dtype=np.float32)
    conv_b = np.asarray(inputs["conv_b"], dtype=np.float32)
    e_w1 = [np.asarray(inputs["e1_w1"], np.float32), np.asarray(inputs["e2_w1"], np.float32)]
    e_b1 = [np.asarray(inputs["e1_b1"], np.float32), np.asarray(inputs["e2_b1"], np.float32)]
    e_w2 = [np.asarray(inputs["e1_w2"], np.float32), np.asarray(inputs["e2_w2"], np.float32)]
    e_b2 = [np.asarray(inputs["e1_b2"], np.float32), np.asarray(inputs["e2_b2"], np.float32)]
    lora_A = [np.asarray(inputs["lora_A1"], np.float32), np.asarray(inputs["lora_A2"], np.float32)]
    lora_B = np.asarray(inputs["lora_B"], np.float32)

    ew1 = np.zeros((33, 256), np.float32)
    for br in range(2):
        ew1[:32, 128 * br:128 * (br + 1)] = e_w1[br].T
        ew1[32, 128 * br:128 * (br + 1)] = e_b1[br]
    w2t = np.concatenate([e_w2[0].T, e_w2[1].T], axis=1).astype(np.float32)
    b2x = np.zeros((16, 2, 16, NB), np.float32)
    for br in range(2):
        b2x[:, br, :, :] = e_b2[br].reshape(16, 16).T[:, :, None]
    b2x = np.ascontiguousarray(b2x.reshape(16, 2 * 16 * NB))
    # G-folded A': A'[q, 6T+i] = sum_kw G43[i,kw] A[q, 3T+kw]
    def gfold(A):
        return np.einsum('ik,qtk->qti', G43, A.reshape(R, 256, 3).astype(np.float64)
                         ).reshape(R, SEG)
    loraAp = np.concatenate([gfold(lora_A[0]), gfold(lora_A[1])], 0).astype(np.float16)
    lba = np.ascontiguousarray(lora_B.reshape(256, 3, 16).transpose(2, 1, 0)
                               ).astype(np.float16)
    # wbasep[p, 6kh+i, j, cout] = sum_kw G43[i,kw] conv_w[cout, 128j+p, kh, kw]
    wb = np.einsum('ik,ocnk->nioc', G43, conv_w.astype(np.float64))  # [kh, i, cout, cin]
    wb = wb.reshape(3, 6, 256, 2, 128).transpose(4, 0, 1, 3, 2)     # [p, kh, i, j, cout]
    wbasep = np.ascontiguousarray(wb.reshape(128, 18, 2, 256)).astype(np.float16)
    convb = np.ascontiguousarray(conv_b.reshape(2, 128).T)

    xpad = np.zeros((B, 2, 128, H + 2, PITCH), dtype=np.float16)
    xpad[:, :, :, 1:H + 1, 1:W + 1] = x.reshape(B, 2, 128, H, W).astype(np.float16)
    # pack each padded row into 4 interleaved segments of 33 (col 4t+s -> seg s)
    xp = np.ascontiguousarray(
        xpad.reshape(B, 2, 128, H + 2, 33, 4).transpose(0, 1, 2, 3, 5, 4)
    ).reshape(B, 2, 128, H + 2, PITCH)
    in_maps = []
    for core in range(NCORES):
        b0 = core * NB
        wmc = np.ones((33, 2 * NB), np.float32)
        for br in range(2):
            for bi in range(NB):
                wmc[:32, NB * br + bi] = wms[br, b0 + bi]
        in_maps.append({
            "x_loc": np.ascontiguousarray(xp[b0:b0 + NB]),
            "wm": wmc, "ew1": ew1, "w2t": w2t, "b2x": b2x,
            "loraAp": loraAp, "lba": lba, "wbasep": wbasep, "convb": convb,
        })
    return in_maps


_NC = None


def kernel(**inputs) -> np.ndarray:
    global _NC
    if _NC is None:
        _NC = _build_nc()
    in_maps = _host_prep(inputs)
    res = run_bass_kernel_spmd(_NC, in_maps, core_ids=list(range(NCORES)))
    return np.concatenate([res.results[c]["out"] for c in range(NCORES)], axis=0)
